# revision 31
# baseline (speedup 1.0000x reference)
"""Trainium2 Bass kernel for nn_PositionalEncoding_61151744360729.

out[b, s, n, :] = x[b, s, n, :] + ||x[b, s+1, n, :] - x[b, s, n, :]||_2
(with distance 0 at s = S-1).

Sharding: data-parallel on batch across 8 NeuronCores (64 batches/core).

Device layout: fp16 end-to-end, c-planar. Host repacks x to, per
(batch, seq-half) partition, [3 coord planes][SH+1 frames][26 nodes]
(nodes padded 25->26 so the one-frame shift is 52B = 4B-aligned and all
DVE tensor_tensor ops hit the 2x perf mode; fp16 I/O halves HBM traffic
vs fp32).

v4 structure (SDMA pool ~55us of transfer is the roofline):
- One batched 3D-AP input DMA per chunk, one output DMA per half-chunk
  piece (the per-DMA ~600-900ns SP sequencer cost made 76 small DMAs a
  near-critical 48us in the original version).
- Input DMAs staggered (chunk k+2 issued at chunk k's compute): the
  SDMA engines round-robin across queued DMAs, so queueing everything
  up front starves chunk 0's completion by ~8us.
- The engines execute their queues in order, so a back-end op whose
  producer hasn't finished stalls everything behind it. With back-end
  ops issued right after the next front (1-piece lag), the serial
  cycle add -> sub -> square -> sqrt (~11us/chunk) dominates: that was
  the old version's real limiter. Here sqrt lags its matmuls by 2
  pieces and add/output lag 5 pieces, so every op's inputs are long
  done when the engine reaches it and engines stream at busy-rate.
- PSUM pieces are [P, 1024] fp32 = 2 banks, 4 in flight, so the deep
  lag fits the 8 PSUM banks.
- Per-plane DVE subtracts (a fused 3-run-AP subtract measured ~10%
  slower than 3 single-run ops), squares on ACT (fused 3-plane op)
  except a DVE plane-2 square on some chunks to balance engines, plane
  sum as identity matmuls into PSUM, ACT sqrt per piece, one stride-0
  broadcast DVE add per piece.
- No GPSIMD tensor ops: the Pool engine shares its SBUF port with the
  DVE, and measured contention inflated concurrent DVE ops 25-100%.
"""

import sys
from contextlib import ExitStack

for _p in ("/opt/trn_rl_repo", "/root/.axon_site/_ro/trn_rl_repo"):
    if _p not in sys.path:
        sys.path.insert(0, _p)

import numpy as np

import concourse.bass as bass
import concourse.tile as tile
from concourse import bacc, mybir
from concourse.bass_utils import run_bass_kernel_spmd

B, S, N, C = 512, 1024, 25, 3
NCORES = 8
BC = B // NCORES           # 64 batches per core
H = 2                      # sequence halves -> 128 partitions
SH = S // H                # 512 frames per half
P = H * BC                 # 128 partitions
NP = 26                    # nodes padded to 26 (4B-aligned frame stride)
IN_PLANE = (SH + 1) * NP   # input elems per coord plane per partition
OUT_PLANE = SH * NP        # output elems per plane per partition
IN_FLAT = P * C * IN_PLANE
OUT_FLAT = P * C * OUT_PLANE
PSUM_W = 512               # one PSUM bank of fp32 per matmul window

# tapered chunk sizes (frames): small head so the first subtract starts
# early off a small DMA, small tail so the final sq->mm->sqrt->add->DMA
# drain chain is short; 64-frame middle keeps per-op overheads low (the
# per-chunk PSUM tile caps chunks at 78 frames)
CHUNKS = [32, 32, 64, 64, 64, 64, 64, 64, 48, 16]
assert sum(CHUNKS) == SH
NCH = len(CHUNKS)
OFF = [sum(CHUNKS[:i]) for i in range(NCH)]
FI_MX = (max(CHUNKS) + 1) * NP     # 1690
FD_MX = max(CHUNKS) * NP           # 1664
PW = 832                           # max piece width (elems per plane)

# square of plane 2 per chunk: 'act' (fused into the 3-plane ACT op) or
# 'dve' (separate DVE multiply) -- tuned so DVE and ACT busy time match
# (measured: fused 3-plane square costs ACT only ~1.1us over the 2-plane
# op while a DVE multiply costs ~1.0us, and DVE is the fuller engine)
# (moving the head chunks' squares fully onto DVE to fill its ramp gaps
# measured SLOWER: DVE binds the stream end-to-end, so extra DVE work
# lands on the end even when inserted into early gaps)
SQ2 = ["act", "act", "act", "dve", "act", "act", "dve", "act", "act",
       "act"]
assert len(SQ2) == NCH

_cache = {}


def _build():
    f16 = mybir.dt.float16
    f32 = mybir.dt.float32
    Af = mybir.ActivationFunctionType
    nc = bacc.Bacc(
        "TRN2", target_bir_lowering=False, debug=False, num_devices=NCORES
    )
    xin = nc.dram_tensor("xin", [IN_FLAT], f16, kind="ExternalInput")
    ident = nc.dram_tensor("ident", [P * P], f16, kind="ExternalInput")
    yout = nc.dram_tensor("yout", [OUT_FLAT], f16, kind="ExternalOutput")

    PF = 2                 # input prefetch depth in chunks

    with tile.TileContext(nc) as tc, ExitStack() as ctx:
        pconst = ctx.enter_context(tc.tile_pool(name="pconst", bufs=2))
        pin0 = ctx.enter_context(tc.tile_pool(name="pin0", bufs=3))
        pin = ctx.enter_context(tc.tile_pool(name="pin", bufs=NCH - 1))
        pd = ctx.enter_context(tc.tile_pool(name="pd", bufs=2))
        ps = ctx.enter_context(tc.tile_pool(name="ps", bufs=5))
        po = ctx.enter_context(tc.tile_pool(name="po", bufs=5))
        ppsum = ctx.enter_context(
            tc.tile_pool(name="ppsum", bufs=2, space="PSUM")
        )

        # chunk 0 inputs first and per plane: they have the SDMA pool to
        # themselves while the SP sequencer works through later issues,
        # so the first subtract can start early
        FI0 = (CHUNKS[0] + 1) * NP
        x0 = []
        for c in range(C):
            t = pin0.tile([P, FI0], f16)
            nc.sync.dma_start(
                t[:],
                bass.AP(xin, c * IN_PLANE, [[C * IN_PLANE, P], [1, FI0]]),
            )
            x0.append(t)
        id_t = pconst.tile([P, P], f16)
        nc.sync.dma_start(id_t[:], bass.AP(ident, 0, [[P, P], [1, P]]))

        # dummy activations so both ACT function tables (square and
        # sqrt) load during the DMA fill instead of mid-stream (each
        # ACT_TABLE_LOAD is ~1.4us on the in-order ACT queue)
        scratch = pconst.tile([P, 2], f16)
        nc.scalar.activation(scratch[:], x0[0][:, 0:2], Af.Sqrt)
        nc.scalar.activation(scratch[:], x0[0][:, 0:2], Af.Square)

        xk = [None] * NCH
        dk = [None] * NCH
        psum_k = [None] * NCH
        dist_k = [None] * NCH

        def issue_in(k):
            fi = (CHUNKS[k] + 1) * NP
            t = pin.tile([P, C * FI_MX], f16)
            src = bass.AP(
                xin,
                OFF[k] * NP,
                [[C * IN_PLANE, P], [IN_PLANE, C], [1, fi]],
            )
            nc.sync.dma_start(t[:, 0:C * fi], src)
            xk[k] = t

        # ALL remaining input DMAs issue before any output DMA enters
        # the SP queue: output issues wait on their adds, and the SP
        # FIFO's head-of-line blocking was stalling later input issues
        # ~9us, which then stalled mid-stream subs. Chunk 0's small
        # per-plane DMAs are already at the queue head, so the SDMA
        # round-robin cannot starve the first chunk (the failure mode
        # of queueing whole 3-plane chunks up front).
        for k in range(1, NCH):
            issue_in(k)

        def x3_view(k):
            fi = (CHUNKS[k] + 1) * NP
            return xk[k][:, 0:C * fi].rearrange("p (c f) -> p c f", c=C)

        def sub_sq(k):
            """per-plane subs + squares for chunk k"""
            fd = CHUNKS[k] * NP
            d_t = pd.tile([P, C * FD_MX], f16)
            dk[k] = d_t
            d3 = d_t[:, 0:C * fd].rearrange("p (c x) -> p c x", c=C)
            if k == 0:
                for c in range(C):
                    nc.vector.tensor_sub(
                        d3[:, c], x0[c][:, NP:NP + fd], x0[c][:, 0:fd]
                    )
            else:
                x3 = x3_view(k)
                for c in range(C):
                    nc.vector.tensor_sub(
                        d3[:, c], x3[:, c, NP:], x3[:, c, 0:fd]
                    )
            if SQ2[k] == "act":
                nc.scalar.activation(
                    d_t[:, 0:C * fd], d_t[:, 0:C * fd], Af.Square
                )
            elif SQ2[k] == "dveall":
                for c in range(C):
                    nc.vector.tensor_mul(d3[:, c], d3[:, c], d3[:, c])
            else:
                nc.scalar.activation(
                    d_t[:, 0:2 * fd], d_t[:, 0:2 * fd], Af.Square
                )
                sq2 = d3[:, 2]
                nc.vector.tensor_mul(sq2, sq2, sq2)

        def mm(k):
            """plane-sum matmuls for chunk k -> [P, 2048] 4-bank psum"""
            fd = CHUNKS[k] * NP
            d3 = dk[k][:, 0:C * fd].rearrange("p (c x) -> p c x", c=C)
            ps_t = ppsum.tile([P, 2048], f32)
            psum_k[k] = ps_t
            for w0 in range(0, fd, PSUM_W):
                w1 = min(w0 + PSUM_W, fd)
                for c in range(C):
                    nc.tensor.matmul(
                        ps_t[:, w0:w1],
                        id_t[:],
                        d3[:, c, w0:w1],
                        start=(c == 0),
                        stop=(c == C - 1),
                    )

        def sqrt_k(k):
            fd = CHUNKS[k] * NP
            s_t = ps.tile([P, FD_MX], f16)
            dist_k[k] = s_t
            nc.scalar.activation(s_t[:, 0:fd], psum_k[k][:, 0:fd], Af.Sqrt)
            psum_k[k] = None

        def add_out(k):
            """broadcast adds + output DMAs for chunk k, per <=832 piece"""
            fd = CHUNKS[k] * NP
            s_t = dist_k[k]
            for lo in range(0, fd, PW):
                w = min(PW, fd - lo)
                hi = lo + w
                o_t = po.tile([P, C * max(w, PW)], f16)
                o3 = o_t[:, 0:C * w].rearrange("p (c x) -> p c x", c=C)
                if k == 0:
                    for c in range(C):
                        nc.vector.tensor_add(
                            o3[:, c], x0[c][:, lo:hi], s_t[:, lo:hi]
                        )
                else:
                    x3 = x3_view(k)
                    sb = (
                        s_t[:, lo:hi]
                        .unsqueeze(1)
                        .broadcast_to([P, C, w])
                    )
                    nc.vector.tensor_add(o3, x3[:, :, lo:hi], sb)
                dst = bass.AP(
                    yout,
                    OFF[k] * NP + lo,
                    [[C * OUT_PLANE, P], [OUT_PLANE, C], [1, w]],
                )
                nc.sync.dma_start(dst, o3)

        # deep-lag software pipeline at chunk granularity: sqrt one
        # chunk behind its matmuls, adds+output DMAs three chunks
        # behind, so every op's inputs are long done when its engine
        # reaches it (lag 4 measured slower: the longer drain outweighs
        # the removed mid-stream coupling stalls)
        for it in range(NCH + 3):
            if it < NCH:
                sub_sq(it)
                mm(it)
            if 0 <= it - 1 < NCH:
                sqrt_k(it - 1)
            if 0 <= it - 3 < NCH:
                add_out(it - 3)

    nc.compile()
    return nc


def kernel(x: np.ndarray, **_unused) -> np.ndarray:
    x = np.asarray(x)
    assert x.shape == (B, S, N, C), x.shape

    if "nc" not in _cache:
        _cache["nc"] = _build()
    nc = _cache["nc"]

    # Host-side repack: fp16, per (batch, half) partition a c-planar
    # [3, SH+1, 26] block; frame SH is the next real frame (half 0) or a
    # copy of the last frame (half 1) so the device-side distance at the
    # true sequence end is exactly 0.
    xh = np.ascontiguousarray(x).astype(np.float16)          # [B,S,25,3]
    ext = np.concatenate([xh, xh[:, -1:]], axis=1)           # [B,S+1,25,3]
    h0 = ext[:, 0:SH + 1]                                    # [B,513,25,3]
    h1 = ext[:, SH:S + 1]                                    # [B,513,25,3]
    hv = np.stack([h0, h1], axis=1)                          # [B,2,513,25,3]
    pl = np.transpose(hv, (0, 1, 4, 2, 3))                   # [B,2,3,513,25]
    buf = np.zeros((B, H, C, SH + 1, NP), np.float16)
    buf[..., :N] = pl

    eye = np.eye(P, dtype=np.float16).reshape(P * P)
    in_maps = [
        {
            "xin": buf[ci * BC:(ci + 1) * BC].reshape(IN_FLAT),
            "ident": eye,
        }
        for ci in range(NCORES)
    ]

    res = run_bass_kernel_spmd(nc, in_maps, core_ids=list(range(NCORES)))
    _cache["last_results"] = res

    out = np.empty((B, S, N, C), dtype=np.float32)
    for ci in range(NCORES):
        y = np.asarray(res.results[ci]["yout"]).reshape(BC, H, C, SH, NP)
        y = y[..., :N]                                       # strip node pad
        y = np.transpose(y, (0, 1, 3, 4, 2))                 # [BC,2,SH,25,3]
        out[ci * BC:(ci + 1) * BC] = y.reshape(BC, S, N, C).astype(np.float32)
    return out


# revision 34
# speedup vs baseline: 1.0233x; 1.0233x over previous
"""Trainium2 Bass kernel for nn_PositionalEncoding_61151744360729.

out[b, s, n, :] = x[b, s, n, :] + ||x[b, s+1, n, :] - x[b, s, n, :]||_2
(with distance 0 at s = S-1).

Sharding: data-parallel on batch across 8 NeuronCores (64 batches/core).

Device layout: fp16 end-to-end, c-planar. Host repacks x to, per
(batch, seq-half) partition, [3 coord planes][SH+1 frames][26 nodes]
(nodes padded 25->26 so the one-frame shift is 52B = 4B-aligned and all
DVE tensor_tensor ops hit the 2x perf mode; fp16 I/O halves HBM traffic
vs fp32).

v4 structure (SDMA pool ~55us of transfer is the roofline):
- One batched 3D-AP input DMA per chunk, one output DMA per half-chunk
  piece (the per-DMA ~600-900ns SP sequencer cost made 76 small DMAs a
  near-critical 48us in the original version).
- Input DMAs staggered (chunk k+2 issued at chunk k's compute): the
  SDMA engines round-robin across queued DMAs, so queueing everything
  up front starves chunk 0's completion by ~8us.
- The engines execute their queues in order, so a back-end op whose
  producer hasn't finished stalls everything behind it. With back-end
  ops issued right after the next front (1-piece lag), the serial
  cycle add -> sub -> square -> sqrt (~11us/chunk) dominates: that was
  the old version's real limiter. Here sqrt lags its matmuls by 2
  pieces and add/output lag 5 pieces, so every op's inputs are long
  done when the engine reaches it and engines stream at busy-rate.
- PSUM pieces are [P, 1024] fp32 = 2 banks, 4 in flight, so the deep
  lag fits the 8 PSUM banks.
- Per-plane DVE subtracts (a fused 3-run-AP subtract measured ~10%
  slower than 3 single-run ops), squares on ACT (fused 3-plane op)
  except a DVE plane-2 square on some chunks to balance engines, plane
  sum as identity matmuls into PSUM, ACT sqrt per piece, one stride-0
  broadcast DVE add per piece.
- No GPSIMD tensor ops: the Pool engine shares its SBUF port with the
  DVE, and measured contention inflated concurrent DVE ops 25-100%.
"""

import sys
from contextlib import ExitStack

for _p in ("/opt/trn_rl_repo", "/root/.axon_site/_ro/trn_rl_repo"):
    if _p not in sys.path:
        sys.path.insert(0, _p)

import numpy as np

import concourse.bass as bass
import concourse.tile as tile
from concourse import bacc, mybir
from concourse.bass_utils import run_bass_kernel_spmd

B, S, N, C = 512, 1024, 25, 3
NCORES = 8
BC = B // NCORES           # 64 batches per core
H = 2                      # sequence halves -> 128 partitions
SH = S // H                # 512 frames per half
P = H * BC                 # 128 partitions
NP = 26                    # nodes padded to 26 (4B-aligned frame stride)
IN_PLANE = (SH + 1) * NP   # input elems per coord plane per partition
OUT_PLANE = SH * NP        # output elems per plane per partition
IN_FLAT = P * C * IN_PLANE
OUT_FLAT = P * C * OUT_PLANE
PSUM_W = 512               # one PSUM bank of fp32 per matmul window

# tapered chunk sizes (frames): small head so the first subtract starts
# early off a small DMA, small tail so the final sq->mm->sqrt->add->DMA
# drain chain is short; 64-frame middle keeps per-op overheads low (the
# per-chunk PSUM tile caps chunks at 78 frames)
CHUNKS = [32, 32, 64, 64, 64, 64, 64, 64, 48, 16]
assert sum(CHUNKS) == SH
NCH = len(CHUNKS)
OFF = [sum(CHUNKS[:i]) for i in range(NCH)]
FI_MX = (max(CHUNKS) + 1) * NP     # 1690
FD_MX = max(CHUNKS) * NP           # 1664
PW = 832                           # max piece width (elems per plane)

# square of plane 2 per chunk: 'act' (fused into the 3-plane ACT op) or
# 'dve' (separate DVE multiply) -- tuned so DVE and ACT busy time match
# (measured: fused 3-plane square costs ACT only ~1.1us over the 2-plane
# op while a DVE multiply costs ~1.0us, and DVE is the fuller engine)
# (moving the head chunks' squares fully onto DVE to fill its ramp gaps
# measured SLOWER: DVE binds the stream end-to-end, so extra DVE work
# lands on the end even when inserted into early gaps)
SQ2 = ["act", "act", "act", "dve", "act", "act", "dve", "act", "act",
       "act"]
assert len(SQ2) == NCH

_cache = {}


def _build():
    f16 = mybir.dt.float16
    f32 = mybir.dt.float32
    Af = mybir.ActivationFunctionType
    nc = bacc.Bacc(
        "TRN2", target_bir_lowering=False, debug=False, num_devices=NCORES
    )
    xin = nc.dram_tensor("xin", [IN_FLAT], f16, kind="ExternalInput")
    ident = nc.dram_tensor("ident", [P * P], f16, kind="ExternalInput")
    yout = nc.dram_tensor("yout", [OUT_FLAT], f16, kind="ExternalOutput")

    PF = 2                 # input prefetch depth in chunks

    with tile.TileContext(nc) as tc, ExitStack() as ctx:
        pconst = ctx.enter_context(tc.tile_pool(name="pconst", bufs=2))
        pin0 = ctx.enter_context(tc.tile_pool(name="pin0", bufs=3))
        pin = ctx.enter_context(tc.tile_pool(name="pin", bufs=7))
        pd = ctx.enter_context(tc.tile_pool(name="pd", bufs=2))
        ps = ctx.enter_context(tc.tile_pool(name="ps", bufs=5))
        po = ctx.enter_context(tc.tile_pool(name="po", bufs=5))
        ppsum = ctx.enter_context(
            tc.tile_pool(name="ppsum", bufs=2, space="PSUM")
        )

        # chunk 0 inputs first and per plane: they have the SDMA pool to
        # themselves while the SP sequencer works through later issues,
        # so the first subtract can start early
        FI0 = (CHUNKS[0] + 1) * NP
        x0 = []
        for c in range(C):
            t = pin0.tile([P, FI0], f16)
            nc.sync.dma_start(
                t[:],
                bass.AP(xin, c * IN_PLANE, [[C * IN_PLANE, P], [1, FI0]]),
            )
            x0.append(t)
        id_t = pconst.tile([P, P], f16)
        nc.sync.dma_start(id_t[:], bass.AP(ident, 0, [[P, P], [1, P]]))

        # dummy activations so both ACT function tables (square and
        # sqrt) load during the DMA fill instead of mid-stream (each
        # ACT_TABLE_LOAD is ~1.4us on the in-order ACT queue)
        scratch = pconst.tile([P, 2], f16)
        nc.scalar.activation(scratch[:], x0[0][:, 0:2], Af.Sqrt)
        nc.scalar.activation(scratch[:], x0[0][:, 0:2], Af.Square)

        xk = [None] * NCH
        dk = [None] * NCH
        psum_k = [None] * NCH
        dist_k = [None] * NCH

        def issue_in(k):
            fi = (CHUNKS[k] + 1) * NP
            t = pin.tile([P, C * FI_MX], f16)
            src = bass.AP(
                xin,
                OFF[k] * NP,
                [[C * IN_PLANE, P], [IN_PLANE, C], [1, fi]],
            )
            nc.sync.dma_start(t[:, 0:C * fi], src)
            xk[k] = t

        # (issuing ALL inputs here before any output DMA enters the SP
        # queue removes SP head-of-line blocking but dilutes the early
        # chunks' SDMA round-robin share — measured net slower)
        for k in range(1, 1 + PF):
            issue_in(k)

        def x3_view(k):
            fi = (CHUNKS[k] + 1) * NP
            return xk[k][:, 0:C * fi].rearrange("p (c f) -> p c f", c=C)

        def sub_sq(k):
            """per-plane subs + squares for chunk k"""
            fd = CHUNKS[k] * NP
            d_t = pd.tile([P, C * FD_MX], f16)
            dk[k] = d_t
            d3 = d_t[:, 0:C * fd].rearrange("p (c x) -> p c x", c=C)
            if k == 0:
                for c in range(C):
                    nc.vector.tensor_sub(
                        d3[:, c], x0[c][:, NP:NP + fd], x0[c][:, 0:fd]
                    )
            else:
                x3 = x3_view(k)
                for c in range(C):
                    nc.vector.tensor_sub(
                        d3[:, c], x3[:, c, NP:], x3[:, c, 0:fd]
                    )
            if SQ2[k] == "act":
                nc.scalar.activation(
                    d_t[:, 0:C * fd], d_t[:, 0:C * fd], Af.Square
                )
            elif SQ2[k] == "dveall":
                for c in range(C):
                    nc.vector.tensor_mul(d3[:, c], d3[:, c], d3[:, c])
            else:
                nc.scalar.activation(
                    d_t[:, 0:2 * fd], d_t[:, 0:2 * fd], Af.Square
                )
                sq2 = d3[:, 2]
                nc.vector.tensor_mul(sq2, sq2, sq2)

        def mm(k):
            """plane-sum matmuls for chunk k -> [P, 2048] 4-bank psum"""
            fd = CHUNKS[k] * NP
            d3 = dk[k][:, 0:C * fd].rearrange("p (c x) -> p c x", c=C)
            ps_t = ppsum.tile([P, 2048], f32)
            psum_k[k] = ps_t
            for w0 in range(0, fd, PSUM_W):
                w1 = min(w0 + PSUM_W, fd)
                for c in range(C):
                    nc.tensor.matmul(
                        ps_t[:, w0:w1],
                        id_t[:],
                        d3[:, c, w0:w1],
                        start=(c == 0),
                        stop=(c == C - 1),
                    )

        def sqrt_k(k):
            fd = CHUNKS[k] * NP
            s_t = ps.tile([P, FD_MX], f16)
            dist_k[k] = s_t
            nc.scalar.activation(s_t[:, 0:fd], psum_k[k][:, 0:fd], Af.Sqrt)
            psum_k[k] = None

        def add_out(k):
            """broadcast adds + output DMAs for chunk k, per <=832 piece"""
            fd = CHUNKS[k] * NP
            s_t = dist_k[k]
            for lo in range(0, fd, PW):
                w = min(PW, fd - lo)
                hi = lo + w
                o_t = po.tile([P, C * max(w, PW)], f16)
                o3 = o_t[:, 0:C * w].rearrange("p (c x) -> p c x", c=C)
                if k == 0:
                    for c in range(C):
                        nc.vector.tensor_add(
                            o3[:, c], x0[c][:, lo:hi], s_t[:, lo:hi]
                        )
                else:
                    x3 = x3_view(k)
                    sb = (
                        s_t[:, lo:hi]
                        .unsqueeze(1)
                        .broadcast_to([P, C, w])
                    )
                    nc.vector.tensor_add(o3, x3[:, :, lo:hi], sb)
                dst = bass.AP(
                    yout,
                    OFF[k] * NP + lo,
                    [[C * OUT_PLANE, P], [OUT_PLANE, C], [1, w]],
                )
                nc.sync.dma_start(dst, o3)

        # deep-lag software pipeline at chunk granularity: sqrt one
        # chunk behind its matmuls, adds+output DMAs three chunks
        # behind, so every op's inputs are long done when its engine
        # reaches it (lag 4 measured slower: the longer drain outweighs
        # the removed mid-stream coupling stalls)
        for it in range(NCH + 3):
            if it < NCH:
                if 1 + PF <= it + PF < NCH:
                    issue_in(it + PF)
                sub_sq(it)
                mm(it)
            if 0 <= it - 1 < NCH:
                sqrt_k(it - 1)
            if 0 <= it - 3 < NCH:
                add_out(it - 3)

    nc.compile()
    return nc


def kernel(x: np.ndarray, **_unused) -> np.ndarray:
    x = np.asarray(x)
    assert x.shape == (B, S, N, C), x.shape

    if "nc" not in _cache:
        _cache["nc"] = _build()
    nc = _cache["nc"]

    # Host-side repack: fp16, per (batch, half) partition a c-planar
    # [3, SH+1, 26] block; frame SH is the next real frame (half 0) or a
    # copy of the last frame (half 1) so the device-side distance at the
    # true sequence end is exactly 0.
    xh = np.ascontiguousarray(x).astype(np.float16)          # [B,S,25,3]
    ext = np.concatenate([xh, xh[:, -1:]], axis=1)           # [B,S+1,25,3]
    h0 = ext[:, 0:SH + 1]                                    # [B,513,25,3]
    h1 = ext[:, SH:S + 1]                                    # [B,513,25,3]
    hv = np.stack([h0, h1], axis=1)                          # [B,2,513,25,3]
    pl = np.transpose(hv, (0, 1, 4, 2, 3))                   # [B,2,3,513,25]
    buf = np.zeros((B, H, C, SH + 1, NP), np.float16)
    buf[..., :N] = pl

    eye = np.eye(P, dtype=np.float16).reshape(P * P)
    in_maps = [
        {
            "xin": buf[ci * BC:(ci + 1) * BC].reshape(IN_FLAT),
            "ident": eye,
        }
        for ci in range(NCORES)
    ]

    res = run_bass_kernel_spmd(nc, in_maps, core_ids=list(range(NCORES)))
    _cache["last_results"] = res

    out = np.empty((B, S, N, C), dtype=np.float32)
    for ci in range(NCORES):
        y = np.asarray(res.results[ci]["yout"]).reshape(BC, H, C, SH, NP)
        y = y[..., :N]                                       # strip node pad
        y = np.transpose(y, (0, 1, 3, 4, 2))                 # [BC,2,SH,25,3]
        out[ci * BC:(ci + 1) * BC] = y.reshape(BC, S, N, C).astype(np.float32)
    return out
